# revision 18
# baseline (speedup 1.0000x reference)
"""MoE gate kernel (EnhancedMoEGate) for 8x Trainium2 NeuronCores.

Computes, for hidden_states [4, 4096, 4096] f32 and gate_weight [64, 4096] f32:
    logits = x @ W.T                       # [T=16384, E=64]
    capped = tanh(logits / 30) * 30
    probs  = softmax(capped)
    routing_weights, selected_experts = top_k(probs, 8); renormalize

Sharding: tokens split 8 ways (2048 tokens/core), gate weight replicated.

Per-core pipeline (default variant dload8v, all fp32):
  - Host pre-transposes x to xd[q, r, t] = x[t, 32q + r] so the four
    per-slice DMA loads are fully contiguous ([128, 32, 512] tiles,
    double-buffered).
  - Per 512-token slice: 32 accumulating fp32 matmuls (lhsT = wt_sb[:,r,:],
    rhs = xt[:, r, slice]) build logitsT [64e, 512t] in PSUM.
  - Fused drain+encode: the PSUM drain overwrites each logit's low 6
    mantissa bits with its expert id ((logit & ~63) | e on the u32 view,
    <= 2^-17 relative perturbation).  max8 alone then yields value AND
    index per token — all 16 DVE max_index instructions are eliminated;
    indices are recovered afterwards with one (q & 63) op.
  - Logits transpose to token-major via a DRAM round trip (2 DMAs through
    a dependency-tracked DRAM tile) instead of 16 PE transposes, then 16
    batched DVE max8 ops.
  - Same batched tanh-softcap epilogue as dload3 (computed on the encoded
    values; the 2^-17 perturbation is far below the 2e-2 tolerance).

Previous default pipeline (dload3):
  - Two interleaved-transposed DMA loads bring x in PRE-TRANSPOSED:
    xt4[q, t, r] = x[t, 32q + r] ([128, 1024, 32] tiles), putting the
    contraction dim on partitions with zero PE transposes.  W.T is loaded
    with the matching interleave: wt_sb[q, r, e] = wt[32q + r, e].
  - 32 accumulating fp32 matmuls per 512-token slice (lhsT = wt_sb[:,r,:],
    rhs = xt4[:, slice, r]) build logitsT [64e, 512t] in PSUM — one
    accumulation group per PSUM pool tile (hardware requirement).
  - Small PE transposes give logits [128t, 64e]; DVE max8/max_index pick the
    top-8 values + indices per token from RAW logits (tanh/softmax are
    monotonic so selection on raw logits matches the reference exactly).
  - Routing weights batched over all 16 token tiles: the full-softmax
    denominator cancels after renormalization, tanh is an odd polynomial on
    DVE, exp on ACT, renormalize via reciprocal+mul.  rw (bitcast) and idx
    share one packed u32 output tensor, split host-side.

Earlier variants (fp32_pack / f32r / dload / dload2) are kept selectable via
MOE_VARIANT for comparison.  On this axon backend wall time is the serial sum
of instruction dispatches, so the design minimizes instruction count: 196
marginal instructions per iteration, of which 128 are the irreducible
matmuls (K<=128 partitions, N<=512 fp32 per PSUM bank).
"""

import os

import numpy as np

T_FULL = 16384
H = 4096
E = 64
TOPK = 8
SOFTCAP = 30.0
N_CORES = 8
T_LOCAL = T_FULL // N_CORES  # 2048
N_TILES = T_LOCAL // 128  # 16 token tiles per core
GROUPS = 4  # groups of 512 tokens
SUBS = 4  # 128-token subtiles per group
CHUNKS = H // 128  # 32 contraction chunks

_CACHE = {}


def _variant():
    # dload8v: slicewise matmuls + encoded top-k + DRAM round-trip transpose
    # (best); dload3 was the previous default; older variants kept for
    # comparison
    return os.environ.get("MOE_VARIANT", "dload8v")


def _build(variant, reps=1):
    import concourse.bass as bass
    import concourse.mybir as mybir
    import concourse.tile as tile
    from concourse import bacc
    from concourse.bass import ts
    from concourse.masks import make_identity
    from contextlib import ExitStack

    f32 = mybir.dt.float32
    f32r = mybir.dt.float32r
    u32 = mybir.dt.uint32

    nox = variant.endswith("_nox")  # bench-only: x stays on device (garbage)
    if nox:
        variant = variant[: -len("_nox")]
    if variant.startswith(("dload7", "dload8", "dload9")):
        return _build7(variant, reps, nox)
    use_f32r = variant.startswith("f32r")
    dload = variant.startswith("dload")  # interleaved transposed DMA loads
    lean = variant.startswith(("dload2", "dload3", "dload4", "dload5"))
    v3 = variant.startswith(("dload3", "dload4", "dload5"))
    v4 = variant.startswith("dload4")  # two-bank accumulator: DEVICE CRASH, unused
    v5 = variant.startswith("dload5")  # logits transpose via DRAM round trip
    pack = (("pack" in variant) or dload) and not lean
    mm_dt = f32r if use_f32r else f32
    R = 32  # h-interleave factor for dload: h = R*q + r

    nc = bacc.Bacc("TRN2", target_bir_lowering=False, debug=False)
    if nox:
        x = nc.dram_tensor("x", [T_LOCAL, H], f32).ap()
    else:
        x = nc.dram_tensor("x", [T_LOCAL, H], f32, kind="ExternalInput").ap()
    wt = nc.dram_tensor("wt", [H, E], f32, kind="ExternalInput").ap()
    if v3:
        # packed output: cols 0-7 = rw bits (f32), cols 8-15 = idx (u32)
        comb = nc.dram_tensor("out", [T_LOCAL, 2 * TOPK], u32, kind="ExternalOutput").ap()
        rw = idx = None
        # v5: token-major DRAM scratch for the logits transpose round trip
        zscratch = nc.dram_tensor("zscratch", [T_LOCAL, E], f32).ap() if v5 else None
    else:
        rw = nc.dram_tensor("rw", [T_LOCAL, TOPK], f32, kind="ExternalOutput").ap()
        idx = nc.dram_tensor("idx", [T_LOCAL, TOPK], u32, kind="ExternalOutput").ap()

    with tile.TileContext(nc) as tc, ExitStack() as ctx:
        consts = ctx.enter_context(tc.tile_pool(name="consts", bufs=1))
        xpool = ctx.enter_context(
            tc.tile_pool(name="xin", bufs=1 if v3 else (2 if dload else 2 * SUBS))
        )
        xtpool = ctx.enter_context(tc.tile_pool(name="xt", bufs=3))
        ppool = ctx.enter_context(tc.tile_pool(name="ptrans", bufs=2, space="PSUM"))
        lgppool = ctx.enter_context(tc.tile_pool(name="plg", bufs=1, space="PSUM"))
        mmpool = ctx.enter_context(tc.tile_pool(name="pmm", bufs=2, space="PSUM"))
        lpool = ctx.enter_context(tc.tile_pool(name="logT", bufs=2))
        epool = ctx.enter_context(tc.tile_pool(name="epi", bufs=3))
        stage = ctx.enter_context(tc.tile_pool(name="stage", bufs=2))

        ident = consts.tile([128, 128], f32)
        make_identity(nc, ident[:])

        if dload:
            # interleaved W.T: wt[R*q + r, e] -> wt_sb[q, r, e]
            wt_sb = consts.tile([128, R, E], f32)
            nc.sync.dma_start(wt_sb[:], wt.rearrange("(q r) e -> q r e", r=R))
        else:
            # W.T chunks: wt[c*128 + p, e] -> wt_sb[p, c, e]
            wt_sb = consts.tile([128, CHUNKS, E], f32)
            nc.sync.dma_start(wt_sb[:], wt.rearrange("(c p) e -> p c e", p=128))
        if use_f32r:
            # float32r operands must be explicitly rounded by their producer
            wt_sb_r = consts.tile([128, CHUNKS, E], f32r)
            nc.vector.tensor_copy(wt_sb_r[:], wt_sb[:])
            wt_mm = wt_sb_r
        else:
            wt_mm = wt_sb

        def _epilogue(top8_all, rw_all):
            # Batched: routing weights from top-8 raw logits.
            # capped = 30*tanh(u/30); rw_k = exp(capped_k)/sum_j exp(capped_j)
            # (the full-softmax denominator cancels after renormalization).
            # tanh(v) = v*(1 + c3 v^2 + c5 v^4 + c7 v^6); |v| <= ~0.2 here so
            # the degree-7 truncation error is ~1e-9.
            F = N_TILES * TOPK  # 128
            u = top8_all[:].rearrange("p a b -> p (a b)")
            c3, c5, c7 = -1.0 / 3.0, 2.0 / 15.0, -17.0 / 315.0
            inv_cap = 1.0 / SOFTCAP

            v = epool.tile([128, F], f32, tag="v")
            h = epool.tile([128, F], f32, tag="h")
            p = epool.tile([128, F], f32, tag="p")
            et = epool.tile([128, N_TILES, TOPK], f32, tag="et")
            s = epool.tile([128, N_TILES, 1], f32, tag="s")

            # Fold 1/30^2 into the coefficients; Horner via
            # scalar_tensor_tensor: b = (b + s) * v each step.
            sc2 = inv_cap * inv_cap
            c3s, c5s, c7s = c3 * sc2, c5 * sc2 * sc2, c7 * sc2 * sc2 * sc2
            nc.vector.tensor_mul(v[:], u, u)  # v = u^2
            nc.vector.scalar_tensor_tensor(
                h[:], v[:], c5s / c7s, v[:],
                op0=mybir.AluOpType.add, op1=mybir.AluOpType.mult,
            )
            nc.vector.scalar_tensor_tensor(
                h[:], h[:], c3s / c7s, v[:],
                op0=mybir.AluOpType.add, op1=mybir.AluOpType.mult,
            )
            # h = h*c7s + 1;  p = h*u = 30*tanh(u/30)
            nc.vector.tensor_scalar(
                h[:], h[:], c7s, 1.0,
                op0=mybir.AluOpType.mult, op1=mybir.AluOpType.add,
            )
            nc.vector.tensor_mul(p[:], h[:], u)
            et_flat = et[:].rearrange("p a b -> p (a b)")
            nc.scalar.activation(et_flat, p[:], mybir.ActivationFunctionType.Exp)
            nc.vector.reduce_sum(s[:], et[:], axis=mybir.AxisListType.X)
            rcp = epool.tile([128, N_TILES, 1], f32, tag="rcp")
            nc.vector.reciprocal(rcp[:], s[:])
            nc.vector.tensor_mul(
                rw_all[:], et[:], rcp[:].to_broadcast([128, N_TILES, TOPK])
            )

        def one_pass():
            top8_all = stage.tile([128, N_TILES, TOPK], f32, tag="top8")
            if v3:
                comb_all = stage.tile([128, N_TILES, 2 * TOPK], u32, tag="comb")
                idx_all = comb_all[:, :, TOPK : 2 * TOPK]
                rw_all = comb_all[:, :, 0:TOPK].bitcast(f32)
            else:
                idx_all = stage.tile([128, N_TILES, TOPK], u32, tag="idxs")
                rw_all = stage.tile([128, N_TILES, TOPK], f32, tag="rws")

            if v3:
                # 2 big interleaved loads; 64 matmuls per load.  v4: both
                # 512-token slices accumulate into one two-bank PSUM tile
                # (each matmul stays within one bank / zero region) so a
                # single DVE copy [64, 1024] drains both.
                for gg in range(2):
                    xt4 = xpool.tile([128, 1024, R], f32, tag="xt4")
                    nc.sync.dma_start(
                        xt4[:],
                        x[ts(gg, 1024), :].rearrange("t (q r) -> q t r", r=R),
                    )
                    if v4:
                        acc = mmpool.tile([128, 1024], f32, tag="acc")
                        for s in range(2):
                            for r in range(R):
                                nc.tensor.matmul(
                                    acc[0:64, ts(s, 512)],
                                    wt_sb[:, r, :],
                                    xt4[:, ts(s, 512), r],
                                    start=(r == 0),
                                    stop=(r == R - 1),
                                )
                        logT = lpool.tile([64, 1024], f32, tag="logT")
                        nc.vector.tensor_copy(logT[:], acc[0:64, :])
                        for j in range(2 * SUBS):
                            n = gg * 2 * SUBS + j
                            lg_ps = lgppool.tile([128, E], f32, tag="lgps")
                            nc.tensor.transpose(
                                lg_ps[:], logT[:, ts(j, 128)], ident[:64, :64]
                            )
                            nc.vector.max(top8_all[:, n, :], lg_ps[:])
                            nc.vector.max_index(
                                idx_all[:, n, :], top8_all[:, n, :], lg_ps[:]
                            )
                        continue
                    for s in range(2):
                        lps = mmpool.tile([128, 512], f32, tag=f"lps{s}")
                        for r in range(R):
                            nc.tensor.matmul(
                                lps[0:64, :],
                                wt_sb[:, r, :],
                                xt4[:, ts(s, 512), r],
                                start=(r == 0),
                                stop=(r == R - 1),
                            )
                        logT = lpool.tile([64, 512], f32)
                        nc.vector.tensor_copy(logT[:], lps[0:64, :])
                        if v5:
                            # strided write into token-major DRAM scratch:
                            # zscratch[t, e] <- logT[e, t-slice]
                            tok0 = (2 * gg + s) * 512
                            nc.sync.dma_start(
                                zscratch[tok0 : tok0 + 512, :].rearrange(
                                    "t e -> e t"
                                ),
                                logT[:],
                            )
                            continue
                        for j in range(SUBS):
                            n = (2 * gg + s) * SUBS + j
                            lg_ps = lgppool.tile([128, E], f32, tag="lgps")
                            nc.tensor.transpose(
                                lg_ps[:], logT[:, ts(j, 128)], ident[:64, :64]
                            )
                            nc.vector.max(top8_all[:, n, :], lg_ps[:])
                            nc.vector.max_index(
                                idx_all[:, n, :], top8_all[:, n, :], lg_ps[:]
                            )
                if v5:
                    # one contiguous-last-dim read back: lg_all[p, n, e]
                    lg_all = lpool.tile([128, N_TILES, E], f32, tag="lgall")
                    nc.sync.dma_start(
                        lg_all[:],
                        zscratch.rearrange("(n p) e -> p n e", p=128),
                    )
                    for n in range(N_TILES):
                        nc.vector.max(top8_all[:, n, :], lg_all[:, n, :])
                        nc.vector.max_index(
                            idx_all[:, n, :], top8_all[:, n, :], lg_all[:, n, :]
                        )
                _epilogue(top8_all, rw_all)
                nc.sync.dma_start(
                    comb.rearrange("(a p) k -> p a k", p=128), comb_all[:]
                )
                return

            for g in range(GROUPS):
                if dload:
                    # One interleaved-transposed DMA per 512-token group:
                    # xt4[q, t, r] = x[512g + t, R*q + r].  Each matmul below
                    # contracts over the stride-R h-subset {R*q + r}.
                    xt4 = xpool.tile([128, 512, R], f32, tag="xt4")
                    nc.sync.dma_start(
                        xt4[:],
                        x[ts(g, 512), :].rearrange("t (q r) -> q t r", r=R),
                    )
                else:
                    xsub = []
                    for j in range(SUBS):
                        xs = xpool.tile([128, H], f32, tag="xs")
                        nc.sync.dma_start(xs[:], x[ts(g * SUBS + j, 128), :])
                        xsub.append(xs)

                # Even chunks accumulate into bank A partitions 0-63 (PE
                # column groups 0-1), odd chunks into bank B partitions
                # 64-127 (column groups 2-3) so the two matmul streams can
                # run concurrently on disjoint column groups of the PE array.
                lpsA = mmpool.tile([128, 512], f32, tag="lpsA")
                if pack:
                    lpsB = mmpool.tile([128, 512], f32, tag="lpsB")
                if dload:
                    for r in range(R):
                        if pack:
                            out_ps = lpsA[0:64, :] if r % 2 == 0 else lpsB[64:128, :]
                            start, stop = r < 2, r >= R - 2
                        else:
                            out_ps = lpsA[0:64, :]
                            start, stop = r == 0, r == R - 1
                        nc.tensor.matmul(
                            out_ps,
                            wt_sb[:, r, :],
                            xt4[:, :, r],
                            start=start,
                            stop=stop,
                        )
                else:
                    for c in range(CHUNKS):
                        xt_ps = ppool.tile([128, 512], f32, tag="xtps")
                        for j in range(SUBS):
                            nc.tensor.transpose(
                                xt_ps[:, ts(j, 128)], xsub[j][:, ts(c, 128)], ident[:]
                            )
                        xt_sb = xtpool.tile([128, 512], mm_dt, tag="xt")
                        nc.vector.tensor_copy(xt_sb[:], xt_ps[:])
                        if pack:
                            out_ps = lpsA[0:64, :] if c % 2 == 0 else lpsB[64:128, :]
                            nc.tensor.matmul(
                                out_ps,
                                wt_mm[:, c, :],
                                xt_sb[:],
                                start=(c < 2),
                                stop=(c >= CHUNKS - 2),
                            )
                        else:
                            nc.tensor.matmul(
                                lpsA[0:64, :],
                                wt_mm[:, c, :],
                                xt_sb[:],
                                start=(c == 0),
                                stop=(c == CHUNKS - 1),
                            )

                # only one DVE input may be PSUM: copy then add
                logT = lpool.tile([64, 512], f32)
                if pack:
                    nc.vector.tensor_copy(logT[:], lpsA[0:64, :])
                    nc.vector.tensor_add(logT[:], logT[:], lpsB[64:128, :])
                else:
                    nc.vector.tensor_copy(logT[:], lpsA[0:64, :])

                for j in range(SUBS):
                    n = g * SUBS + j
                    lg_ps = lgppool.tile([128, E], f32, tag="lgps")
                    nc.tensor.transpose(lg_ps[:], logT[:, ts(j, 128)], ident[:64, :64])
                    if dload:
                        # max8/max_index read straight from PSUM (1 PSUM input)
                        nc.vector.max(top8_all[:, n, :], lg_ps[:])
                        nc.vector.max_index(idx_all[:, n, :], top8_all[:, n, :], lg_ps[:])
                    else:
                        lg_sb = epool.tile([128, E], f32, tag="lg")
                        nc.vector.tensor_copy(lg_sb[:], lg_ps[:])
                        nc.vector.max(top8_all[:, n, :], lg_sb[:])
                        nc.vector.max_index(idx_all[:, n, :], top8_all[:, n, :], lg_sb[:])

            # Batched epilogue: routing weights from top-8 raw logits.
            # capped = 30*tanh(u/30); rw_k = exp(capped_k)/sum_j exp(capped_j)
            # (the full-softmax denominator cancels after renormalization).
            # tanh(v) = v*(1 + c3 v^2 + c5 v^4 + c7 v^6); |v| <= ~0.2 here so
            # the degree-7 truncation error is ~1e-9.
            F = N_TILES * TOPK  # 128
            u = top8_all[:].rearrange("p a b -> p (a b)")
            c3, c5, c7 = -1.0 / 3.0, 2.0 / 15.0, -17.0 / 315.0
            inv_cap = 1.0 / SOFTCAP

            v = epool.tile([128, F], f32, tag="v")
            h = epool.tile([128, F], f32, tag="h")
            p = epool.tile([128, F], f32, tag="p")
            et = epool.tile([128, N_TILES, TOPK], f32, tag="et")
            s = epool.tile([128, N_TILES, 1], f32, tag="s")
            r = epool.tile([128, N_TILES, 1], f32, tag="r")

            if lean:
                # Fold 1/30^2 into the coefficients; Horner via
                # scalar_tensor_tensor: b = (b + s) * v each step.
                sc2 = inv_cap * inv_cap
                c3s, c5s, c7s = c3 * sc2, c5 * sc2 * sc2, c7 * sc2 * sc2 * sc2
                nc.vector.tensor_mul(v[:], u, u)  # v = u^2
                nc.vector.scalar_tensor_tensor(
                    h[:], v[:], c5s / c7s, v[:],
                    op0=mybir.AluOpType.add, op1=mybir.AluOpType.mult,
                )
                nc.vector.scalar_tensor_tensor(
                    h[:], h[:], c3s / c7s, v[:],
                    op0=mybir.AluOpType.add, op1=mybir.AluOpType.mult,
                )
                # h = h*c7s + 1;  p = h*u = 30*tanh(u/30)
                nc.vector.tensor_scalar(
                    h[:], h[:], c7s, 1.0,
                    op0=mybir.AluOpType.mult, op1=mybir.AluOpType.add,
                )
                nc.vector.tensor_mul(p[:], h[:], u)
            else:
                # v = (u/30)^2
                nc.vector.tensor_mul(v[:], u, u)
                nc.vector.tensor_scalar_mul(v[:], v[:], inv_cap * inv_cap)
                # h = ((c7 v + c5) v + c3) v + 1
                nc.vector.tensor_scalar(
                    h[:], v[:], c7, c5, op0=mybir.AluOpType.mult, op1=mybir.AluOpType.add
                )
                nc.vector.tensor_mul(h[:], h[:], v[:])
                nc.vector.tensor_scalar_add(h[:], h[:], c3)
                nc.vector.tensor_mul(h[:], h[:], v[:])
                nc.vector.tensor_scalar_add(h[:], h[:], 1.0)
                # p = u * h = 30*tanh(u/30); et = exp(p)
                nc.vector.tensor_mul(p[:], h[:], u)
            et_flat = et[:].rearrange("p a b -> p (a b)")
            nc.scalar.activation(et_flat, p[:], mybir.ActivationFunctionType.Exp)
            nc.vector.reduce_sum(s[:], et[:], axis=mybir.AxisListType.X)
            nc.vector.reciprocal(r[:], s[:])
            nc.vector.tensor_mul(
                rw_all[:], et[:], r[:].to_broadcast([128, N_TILES, TOPK])
            )

            nc.sync.dma_start(rw.rearrange("(a p) k -> p a k", p=128), rw_all[:])
            nc.sync.dma_start(idx.rearrange("(a p) k -> p a k", p=128), idx_all[:])

        for _ in range(reps):
            one_pass()

    nc.compile()
    return nc


def _build7(variant, reps, nox):
    """dload7: mantissa-encoded top-k (no max_index), fused encode+drain,
    host-pretransposed contiguous input loads, weight-reuse matmul order.

    Index encoding: logits' low 6 mantissa bits are overwritten with the
    expert id BEFORE selection (q = (logit & ~63) | e on the u32 view).
    This perturbs each logit by <= 2^-17 relative -- far below the top-8
    decision margins -- and makes max8 return value+index in one op; the
    index is recovered by (q & 63).  Eliminates all 16 max_index
    instructions.

    Transpose routes: "-pe" = 16 PE transposes (default), "-v5" = DRAM
    round trip (2 DMAs).
    """
    import concourse.bass as bass
    import concourse.mybir as mybir
    import concourse.tile as tile
    from concourse import bacc
    from concourse.bass import ts
    from concourse.masks import make_identity
    from contextlib import ExitStack

    f32 = mybir.dt.float32
    u32 = mybir.dt.uint32
    route = "v5" if "-v5" in variant else "pe"
    slicewise = variant.startswith(("dload8", "dload9"))
    quarters = slicewise and ("q" in variant[6:] or variant.startswith("dload9"))
    pair = variant.startswith("dload9")
    v5route = slicewise and "v" in variant[6:]

    nc = bacc.Bacc("TRN2", target_bir_lowering=False, debug=False)
    if nox:
        x = nc.dram_tensor("x", [128, 32, T_LOCAL], f32).ap()
    else:
        x = nc.dram_tensor("x", [128, 32, T_LOCAL], f32, kind="ExternalInput").ap()
    wt = nc.dram_tensor("wt", [H, E], f32, kind="ExternalInput").ap()
    cmask_d = nc.dram_tensor("cmask", [128, 4224], u32, kind="ExternalInput").ap()
    comb = nc.dram_tensor("out", [T_LOCAL, 2 * TOPK], u32, kind="ExternalOutput").ap()
    use_zs = route == "v5" or (slicewise and "v" in variant[6:])

    with tile.TileContext(nc) as tc, ExitStack() as ctx:
        consts = ctx.enter_context(tc.tile_pool(name="consts", bufs=1))
        xpool = ctx.enter_context(
            tc.tile_pool(name="xin", bufs=2 if (slicewise and quarters) else 1)
        )
        mmpool = ctx.enter_context(
            tc.tile_pool(name="pmm", bufs=2 if slicewise else 1, space="PSUM")
        )
        lpool = ctx.enter_context(tc.tile_pool(name="logT", bufs=2))
        epool = ctx.enter_context(tc.tile_pool(name="epi", bufs=3))
        stage = ctx.enter_context(tc.tile_pool(name="stage", bufs=2))
        if use_zs:
            zpool = ctx.enter_context(tc.tile_pool(name="zs", bufs=2, space="DRAM"))
        need_pe = (route == "pe") and not (slicewise and "v" in variant[6:])
        if need_pe:
            lgppool = ctx.enter_context(
                tc.tile_pool(name="plg", bufs=1 if slicewise else 4, space="PSUM")
            )

        wt_sb = consts.tile([128, 32, E], f32)
        nc.sync.dma_start(wt_sb[:], wt.rearrange("(q r) e -> q r e", r=32))
        cmask = consts.tile([128, 4224], u32)
        nc.sync.dma_start(cmask[:], cmask_d)
        andm = cmask[0:64, 0:2048]
        orm = cmask[0:64, 2048:4096]
        m63 = cmask[:, 4096:4224].rearrange("p (a k) -> p a k", k=TOPK)
        if need_pe:
            ident = consts.tile([128, 128], f32)
            make_identity(nc, ident[:])

        def one_pass(rep_i):
            comb_all = stage.tile([128, N_TILES, 2 * TOPK], u32, tag="comb")
            idx_all = comb_all[:, :, TOPK : 2 * TOPK]
            rw_all = comb_all[:, :, 0:TOPK].bitcast(f32)
            top8q = stage.tile([128, N_TILES, TOPK], f32, tag="top8")

            if slicewise:
                # dload8*: dload3's proven slice-by-slice structure, with the
                # fused encode-drain and no max_index.
                # q: quarter loads (double-buffered); p: pair r-outer mm;
                # v: DRAM round-trip transpose instead of PE transposes.
                logT_all = (
                    lpool.tile([64, 2048], u32, tag="logTall", name="logT_all")
                    if v5route
                    else None
                )

                def encode(src_ps, g):
                    tmp_u = lpool.tile([64, 512], u32, tag="tmp")
                    if v5route:
                        logT_u = logT_all[:, ts(g, 512)]
                    else:
                        logT_t = lpool.tile([64, 512], u32, tag="logT")
                        logT_u = logT_t[:]
                    nc.vector.tensor_tensor(
                        tmp_u[:],
                        src_ps[0:64, :].bitcast(u32),
                        andm[:, 0:512],
                        op=mybir.AluOpType.bitwise_and,
                    )
                    nc.vector.tensor_tensor(
                        logT_u, tmp_u[:], orm[:, 0:512], op=mybir.AluOpType.bitwise_or
                    )
                    return logT_u

                def select(logT_u, g):
                    logT_f = logT_u.bitcast(f32)
                    for j in range(SUBS):
                        n = g * SUBS + j
                        lg_ps = lgppool.tile([128, E], f32, tag="lgps")
                        nc.tensor.transpose(
                            lg_ps[:], logT_f[:, ts(j, 128)], ident[0:64, 0:64]
                        )
                        nc.vector.max(top8q[:, n, :], lg_ps[:])

                if pair:
                    for h in range(2):
                        xts, lpss = [], []
                        for s in range(2):
                            xt = xpool.tile([128, 32, 512], f32, tag="xt")
                            nc.sync.dma_start(xt[:], x[:, :, ts(2 * h + s, 512)])
                            xts.append(xt)
                            lpss.append(
                                mmpool.tile(
                                    [128, 512], f32, tag=f"lps{s}", name=f"lps{s}"
                                )
                            )
                        for r in range(32):
                            for s in range(2):
                                nc.tensor.matmul(
                                    lpss[s][0:64, :],
                                    wt_sb[:, r, :],
                                    xts[s][:, r, :],
                                    start=(r == 0),
                                    stop=(r == 31),
                                )
                        for s in range(2):
                            g = 2 * h + s
                            lu = encode(lpss[s], g)
                            if not v5route:
                                select(lu, g)
                else:
                    for g in range(4):
                        if quarters:
                            xt = xpool.tile([128, 32, 512], f32, tag="xt")
                            nc.sync.dma_start(xt[:], x[:, :, ts(g, 512)])
                            rhs = xt
                        else:
                            if g % 2 == 0:
                                xth = xpool.tile([128, 32, 1024], f32, tag="xt")
                                nc.sync.dma_start(
                                    xth[:], x[:, :, ts(g // 2, 1024)]
                                )
                            rhs = None
                        lps = mmpool.tile([128, 512], f32, tag=f"lps{g % 2}")
                        for r in range(32):
                            nc.tensor.matmul(
                                lps[0:64, :],
                                wt_sb[:, r, :],
                                xt[:, r, :]
                                if quarters
                                else xth[:, r, ts(g % 2, 512)],
                                start=(r == 0),
                                stop=(r == 31),
                            )
                        lu = encode(lps, g)
                        if not v5route:
                            select(lu, g)

                if v5route:
                    zs_t = zpool.tile([T_LOCAL, E], u32, tag="zs", name="zs_t")
                    zs = zs_t
                    nc.sync.dma_start(zs.rearrange("t e -> e t"), logT_all[:])
                    lg_all = lpool.tile([128, N_TILES, E], u32, tag="lgall")
                    nc.sync.dma_start(
                        lg_all[:], zs.rearrange("(a p) e -> p a e", p=128)
                    )
                    for n in range(N_TILES):
                        nc.vector.max(top8q[:, n, :], lg_all.bitcast(f32)[:, n, :])
            else:
                # dload7: 2 halves, r-outer over the half's 2 token groups
                acc = mmpool.tile([128, 2048], f32, tag="acc")
                for h in range(2):
                    xt = xpool.tile([128, 32, 1024], f32, tag="xt")
                    nc.sync.dma_start(xt[:], x[:, :, ts(h, 1024)])
                    for r in range(32):
                        for s in range(2):
                            g = 2 * h + s
                            nc.tensor.matmul(
                                acc[0:64, ts(g, 512)],
                                wt_sb[:, r, :],
                                xt[:, r, ts(s, 512)],
                                start=(r == 0),
                                stop=(r == 31),
                                skip_group_check=True,
                            )

                # fused drain + index-encode: logT = (acc & ~63) | e
                tmp_u = lpool.tile([64, 2048], u32, tag="tmp")
                logT_u = lpool.tile([64, 2048], u32, tag="logT")
                nc.vector.tensor_tensor(
                    tmp_u[:],
                    acc[0:64, :].bitcast(u32),
                    andm,
                    op=mybir.AluOpType.bitwise_and,
                )
                nc.vector.tensor_tensor(
                    logT_u[:], tmp_u[:], orm, op=mybir.AluOpType.bitwise_or
                )

                if route == "v5":
                    zs_t = zpool.tile([T_LOCAL, E], u32, tag="zs", name="zs_t")
                    zs = zs_t
                    nc.sync.dma_start(zs.rearrange("t e -> e t"), logT_u[:])
                    lg_all = lpool.tile([128, N_TILES, E], u32, tag="lgall")
                    nc.sync.dma_start(
                        lg_all[:], zs.rearrange("(a p) e -> p a e", p=128)
                    )
                    for n in range(N_TILES):
                        nc.vector.max(top8q[:, n, :], lg_all.bitcast(f32)[:, n, :])
                else:
                    logT_f = logT_u[:].bitcast(f32)
                    for j4 in range(4):
                        lgps = []
                        for jj in range(4):
                            j = 4 * j4 + jj
                            lg_ps = lgppool.tile([128, E], f32, tag="lgps")
                            nc.tensor.transpose(
                                lg_ps[:], logT_f[:, ts(j, 128)], ident[0:64, 0:64]
                            )
                            lgps.append(lg_ps)
                        for jj in range(4):
                            n = 4 * j4 + jj
                            nc.vector.max(top8q[:, n, :], lgps[jj][:])

            # index decode: idx = q & 63
            nc.vector.tensor_tensor(
                idx_all,
                top8q[:].bitcast(u32),
                m63,
                op=mybir.AluOpType.bitwise_and,
            )

            # routing weights from encoded top-8 logits (see dload3 epilogue)
            F = N_TILES * TOPK
            u = top8q[:].rearrange("p a b -> p (a b)")
            c3, c5, c7 = -1.0 / 3.0, 2.0 / 15.0, -17.0 / 315.0
            sc2 = (1.0 / SOFTCAP) ** 2
            c3s, c5s, c7s = c3 * sc2, c5 * sc2 * sc2, c7 * sc2 * sc2 * sc2

            v = epool.tile([128, F], f32, tag="v")
            hh = epool.tile([128, F], f32, tag="hh")
            p = epool.tile([128, F], f32, tag="p")
            et = epool.tile([128, N_TILES, TOPK], f32, tag="et")
            s_ = epool.tile([128, N_TILES, 1], f32, tag="s")
            rcp = epool.tile([128, N_TILES, 1], f32, tag="rcp")

            nc.vector.tensor_mul(v[:], u, u)
            nc.vector.scalar_tensor_tensor(
                hh[:], v[:], c5s / c7s, v[:],
                op0=mybir.AluOpType.add, op1=mybir.AluOpType.mult,
            )
            nc.vector.scalar_tensor_tensor(
                hh[:], hh[:], c3s / c7s, v[:],
                op0=mybir.AluOpType.add, op1=mybir.AluOpType.mult,
            )
            nc.vector.tensor_scalar(
                hh[:], hh[:], c7s, 1.0,
                op0=mybir.AluOpType.mult, op1=mybir.AluOpType.add,
            )
            nc.vector.tensor_mul(p[:], hh[:], u)
            et_flat = et[:].rearrange("p a b -> p (a b)")
            nc.scalar.activation(et_flat, p[:], mybir.ActivationFunctionType.Exp)
            nc.vector.reduce_sum(s_[:], et[:], axis=mybir.AxisListType.X)
            nc.vector.reciprocal(rcp[:], s_[:])
            nc.vector.tensor_mul(
                rw_all[:], et[:], rcp[:].to_broadcast([128, N_TILES, TOPK])
            )

            nc.sync.dma_start(
                comb.rearrange("(a p) k -> p a k", p=128), comb_all[:]
            )

        for rep_i in range(reps):
            one_pass(rep_i)

    nc.compile()
    return nc


def _aux_inputs(variant):
    """Constant auxiliary inputs required by a variant (beyond x/wt)."""
    if not variant.startswith(("dload7", "dload8", "dload9")):
        return {}
    cm = np.zeros((128, 4224), np.uint32)
    cm[0:64, 0:2048] = np.uint32(0xFFFFFFC0)
    cm[0:64, 2048:4096] = np.arange(64, dtype=np.uint32)[:, None]
    cm[:, 4096:4224] = 63
    return {"cmask": cm}


def _get_nc(variant, reps=1):
    key = (variant, reps)
    if key not in _CACHE:
        _CACHE[key] = _build(variant, reps)
    return _CACHE[key]


def kernel(hidden_states, gate_weight):
    from concourse.bass_utils import run_bass_kernel_spmd

    x = np.ascontiguousarray(np.asarray(hidden_states, dtype=np.float32)).reshape(
        T_FULL, H
    )
    w = np.asarray(gate_weight, dtype=np.float32)
    wt = np.ascontiguousarray(w.T)

    variant = _variant()
    nc = _get_nc(variant, int(os.environ.get("MOE_REPS", "1")))
    aux = _aux_inputs(variant)
    if variant.startswith(("dload7", "dload8", "dload9")):
        # host pre-transpose: xd[q, r, t] = x[t, 32q + r] (contiguous loads)
        in_maps = [
            {
                "x": np.ascontiguousarray(
                    x[i * T_LOCAL : (i + 1) * T_LOCAL]
                    .reshape(T_LOCAL, 128, 32)
                    .transpose(1, 2, 0)
                ),
                "wt": wt,
                **aux,
            }
            for i in range(N_CORES)
        ]
    else:
        in_maps = [
            {"x": np.ascontiguousarray(x[i * T_LOCAL : (i + 1) * T_LOCAL]), "wt": wt}
            for i in range(N_CORES)
        ]
    kwargs = {}
    if os.environ.get("MOE_TRACE"):
        kwargs["trace"] = True
        if os.environ.get("MOE_TMPDIR"):
            kwargs["tmpdir"] = os.environ["MOE_TMPDIR"]
    res = run_bass_kernel_spmd(nc, in_maps, core_ids=list(range(N_CORES)), **kwargs)
    if os.environ.get("MOE_TRACE"):
        kernel.last_exec_time_ns = res.exec_time_ns
        kernel.last_profile = res.profile_json
    if variant.startswith(("dload3", "dload4", "dload5", "dload7", "dload8", "dload9")):
        out = np.concatenate([res.results[i]["out"] for i in range(N_CORES)], axis=0)
        rw = np.ascontiguousarray(out[:, :TOPK]).view(np.float32)
        idx = out[:, TOPK:]
    else:
        rw = np.concatenate([res.results[i]["rw"] for i in range(N_CORES)], axis=0)
        idx = np.concatenate([res.results[i]["idx"] for i in range(N_CORES)], axis=0)
    return rw.astype(np.float32), idx.astype(np.int32)



# revision 19
# speedup vs baseline: 1.2045x; 1.2045x over previous
"""MoE gate kernel (EnhancedMoEGate) for 8x Trainium2 NeuronCores.

Computes, for hidden_states [4, 4096, 4096] f32 and gate_weight [64, 4096] f32:
    logits = x @ W.T                       # [T=16384, E=64]
    capped = tanh(logits / 30) * 30
    probs  = softmax(capped)
    routing_weights, selected_experts = top_k(probs, 8); renormalize

Sharding: tokens split 8 ways (2048 tokens/core), gate weight replicated.

Per-core pipeline (default variant dload8v, all fp32):
  - Host pre-transposes x to xd[q, r, t] = x[t, 32q + r] so the four
    per-slice DMA loads are fully contiguous ([128, 32, 512] tiles,
    double-buffered).
  - Per 512-token slice: 32 accumulating fp32 matmuls (lhsT = wt_sb[:,r,:],
    rhs = xt[:, r, slice]) build logitsT [64e, 512t] in PSUM.
  - Fused drain+encode: the PSUM drain overwrites each logit's low 6
    mantissa bits with its expert id ((logit & ~63) | e on the u32 view,
    <= 2^-17 relative perturbation).  max8 alone then yields value AND
    index per token — all 16 DVE max_index instructions are eliminated;
    indices are recovered afterwards with one (q & 63) op.
  - Logits transpose to token-major via a DRAM round trip (2 DMAs through
    a dependency-tracked DRAM tile) instead of 16 PE transposes, then 16
    batched DVE max8 ops.
  - Same batched tanh-softcap epilogue as dload3 (computed on the encoded
    values; the 2^-17 perturbation is far below the 2e-2 tolerance).

Previous default pipeline (dload3):
  - Two interleaved-transposed DMA loads bring x in PRE-TRANSPOSED:
    xt4[q, t, r] = x[t, 32q + r] ([128, 1024, 32] tiles), putting the
    contraction dim on partitions with zero PE transposes.  W.T is loaded
    with the matching interleave: wt_sb[q, r, e] = wt[32q + r, e].
  - 32 accumulating fp32 matmuls per 512-token slice (lhsT = wt_sb[:,r,:],
    rhs = xt4[:, slice, r]) build logitsT [64e, 512t] in PSUM — one
    accumulation group per PSUM pool tile (hardware requirement).
  - Small PE transposes give logits [128t, 64e]; DVE max8/max_index pick the
    top-8 values + indices per token from RAW logits (tanh/softmax are
    monotonic so selection on raw logits matches the reference exactly).
  - Routing weights batched over all 16 token tiles: the full-softmax
    denominator cancels after renormalization, tanh is an odd polynomial on
    DVE, exp on ACT, renormalize via reciprocal+mul.  rw (bitcast) and idx
    share one packed u32 output tensor, split host-side.

Earlier variants (fp32_pack / f32r / dload / dload2) are kept selectable via
MOE_VARIANT for comparison.  On this axon backend wall time is the serial sum
of instruction dispatches, so the design minimizes instruction count: 196
marginal instructions per iteration, of which 128 are the irreducible
matmuls (K<=128 partitions, N<=512 fp32 per PSUM bank).
"""

import os

import numpy as np

T_FULL = 16384
H = 4096
E = 64
TOPK = 8
SOFTCAP = 30.0
N_CORES = 8
T_LOCAL = T_FULL // N_CORES  # 2048
N_TILES = T_LOCAL // 128  # 16 token tiles per core
GROUPS = 4  # groups of 512 tokens
SUBS = 4  # 128-token subtiles per group
CHUNKS = H // 128  # 32 contraction chunks

_CACHE = {}


def _variant():
    # dload8v: slicewise matmuls + encoded top-k + DRAM round-trip transpose
    # (best); dload3 was the previous default; older variants kept for
    # comparison
    return os.environ.get("MOE_VARIANT", "dload8v")


def _build(variant, reps=1):
    import concourse.bass as bass
    import concourse.mybir as mybir
    import concourse.tile as tile
    from concourse import bacc
    from concourse.bass import ts
    from concourse.masks import make_identity
    from contextlib import ExitStack

    f32 = mybir.dt.float32
    f32r = mybir.dt.float32r
    u32 = mybir.dt.uint32

    nox = variant.endswith("_nox")  # bench-only: x stays on device (garbage)
    if nox:
        variant = variant[: -len("_nox")]
    if variant.startswith(("dload7", "dload8", "dload9")):
        return _build7(variant, reps, nox)
    use_f32r = variant.startswith("f32r")
    dload = variant.startswith("dload")  # interleaved transposed DMA loads
    lean = variant.startswith(("dload2", "dload3", "dload4", "dload5"))
    v3 = variant.startswith(("dload3", "dload4", "dload5"))
    v4 = variant.startswith("dload4")  # two-bank accumulator: DEVICE CRASH, unused
    v5 = variant.startswith("dload5")  # logits transpose via DRAM round trip
    pack = (("pack" in variant) or dload) and not lean
    mm_dt = f32r if use_f32r else f32
    R = 32  # h-interleave factor for dload: h = R*q + r

    nc = bacc.Bacc("TRN2", target_bir_lowering=False, debug=False)
    if nox:
        x = nc.dram_tensor("x", [T_LOCAL, H], f32).ap()
    else:
        x = nc.dram_tensor("x", [T_LOCAL, H], f32, kind="ExternalInput").ap()
    wt = nc.dram_tensor("wt", [H, E], f32, kind="ExternalInput").ap()
    if v3:
        # packed output: cols 0-7 = rw bits (f32), cols 8-15 = idx (u32)
        comb = nc.dram_tensor("out", [T_LOCAL, 2 * TOPK], u32, kind="ExternalOutput").ap()
        rw = idx = None
        # v5: token-major DRAM scratch for the logits transpose round trip
        zscratch = nc.dram_tensor("zscratch", [T_LOCAL, E], f32).ap() if v5 else None
    else:
        rw = nc.dram_tensor("rw", [T_LOCAL, TOPK], f32, kind="ExternalOutput").ap()
        idx = nc.dram_tensor("idx", [T_LOCAL, TOPK], u32, kind="ExternalOutput").ap()

    with tile.TileContext(nc) as tc, ExitStack() as ctx:
        consts = ctx.enter_context(tc.tile_pool(name="consts", bufs=1))
        xpool = ctx.enter_context(
            tc.tile_pool(name="xin", bufs=1 if v3 else (2 if dload else 2 * SUBS))
        )
        xtpool = ctx.enter_context(tc.tile_pool(name="xt", bufs=3))
        ppool = ctx.enter_context(tc.tile_pool(name="ptrans", bufs=2, space="PSUM"))
        lgppool = ctx.enter_context(tc.tile_pool(name="plg", bufs=1, space="PSUM"))
        mmpool = ctx.enter_context(tc.tile_pool(name="pmm", bufs=2, space="PSUM"))
        lpool = ctx.enter_context(tc.tile_pool(name="logT", bufs=2))
        epool = ctx.enter_context(tc.tile_pool(name="epi", bufs=3))
        stage = ctx.enter_context(tc.tile_pool(name="stage", bufs=2))

        ident = consts.tile([128, 128], f32)
        make_identity(nc, ident[:])

        if dload:
            # interleaved W.T: wt[R*q + r, e] -> wt_sb[q, r, e]
            wt_sb = consts.tile([128, R, E], f32)
            nc.sync.dma_start(wt_sb[:], wt.rearrange("(q r) e -> q r e", r=R))
        else:
            # W.T chunks: wt[c*128 + p, e] -> wt_sb[p, c, e]
            wt_sb = consts.tile([128, CHUNKS, E], f32)
            nc.sync.dma_start(wt_sb[:], wt.rearrange("(c p) e -> p c e", p=128))
        if use_f32r:
            # float32r operands must be explicitly rounded by their producer
            wt_sb_r = consts.tile([128, CHUNKS, E], f32r)
            nc.vector.tensor_copy(wt_sb_r[:], wt_sb[:])
            wt_mm = wt_sb_r
        else:
            wt_mm = wt_sb

        def _epilogue(top8_all, rw_all):
            # Batched: routing weights from top-8 raw logits.
            # capped = 30*tanh(u/30); rw_k = exp(capped_k)/sum_j exp(capped_j)
            # (the full-softmax denominator cancels after renormalization).
            # tanh(v) = v*(1 + c3 v^2 + c5 v^4 + c7 v^6); |v| <= ~0.2 here so
            # the degree-7 truncation error is ~1e-9.
            F = N_TILES * TOPK  # 128
            u = top8_all[:].rearrange("p a b -> p (a b)")
            c3, c5, c7 = -1.0 / 3.0, 2.0 / 15.0, -17.0 / 315.0
            inv_cap = 1.0 / SOFTCAP

            v = epool.tile([128, F], f32, tag="v")
            h = epool.tile([128, F], f32, tag="h")
            p = epool.tile([128, F], f32, tag="p")
            et = epool.tile([128, N_TILES, TOPK], f32, tag="et")
            s = epool.tile([128, N_TILES, 1], f32, tag="s")

            # Fold 1/30^2 into the coefficients; Horner via
            # scalar_tensor_tensor: b = (b + s) * v each step.
            sc2 = inv_cap * inv_cap
            c3s, c5s, c7s = c3 * sc2, c5 * sc2 * sc2, c7 * sc2 * sc2 * sc2
            nc.vector.tensor_mul(v[:], u, u)  # v = u^2
            nc.vector.scalar_tensor_tensor(
                h[:], v[:], c5s / c7s, v[:],
                op0=mybir.AluOpType.add, op1=mybir.AluOpType.mult,
            )
            nc.vector.scalar_tensor_tensor(
                h[:], h[:], c3s / c7s, v[:],
                op0=mybir.AluOpType.add, op1=mybir.AluOpType.mult,
            )
            # h = h*c7s + 1;  p = h*u = 30*tanh(u/30)
            nc.vector.tensor_scalar(
                h[:], h[:], c7s, 1.0,
                op0=mybir.AluOpType.mult, op1=mybir.AluOpType.add,
            )
            nc.vector.tensor_mul(p[:], h[:], u)
            et_flat = et[:].rearrange("p a b -> p (a b)")
            nc.scalar.activation(et_flat, p[:], mybir.ActivationFunctionType.Exp)
            nc.vector.reduce_sum(s[:], et[:], axis=mybir.AxisListType.X)
            rcp = epool.tile([128, N_TILES, 1], f32, tag="rcp")
            nc.vector.reciprocal(rcp[:], s[:])
            nc.vector.tensor_mul(
                rw_all[:], et[:], rcp[:].to_broadcast([128, N_TILES, TOPK])
            )

        def one_pass():
            top8_all = stage.tile([128, N_TILES, TOPK], f32, tag="top8")
            if v3:
                comb_all = stage.tile([128, N_TILES, 2 * TOPK], u32, tag="comb")
                idx_all = comb_all[:, :, TOPK : 2 * TOPK]
                rw_all = comb_all[:, :, 0:TOPK].bitcast(f32)
            else:
                idx_all = stage.tile([128, N_TILES, TOPK], u32, tag="idxs")
                rw_all = stage.tile([128, N_TILES, TOPK], f32, tag="rws")

            if v3:
                # 2 big interleaved loads; 64 matmuls per load.  v4: both
                # 512-token slices accumulate into one two-bank PSUM tile
                # (each matmul stays within one bank / zero region) so a
                # single DVE copy [64, 1024] drains both.
                for gg in range(2):
                    xt4 = xpool.tile([128, 1024, R], f32, tag="xt4")
                    nc.sync.dma_start(
                        xt4[:],
                        x[ts(gg, 1024), :].rearrange("t (q r) -> q t r", r=R),
                    )
                    if v4:
                        acc = mmpool.tile([128, 1024], f32, tag="acc")
                        for s in range(2):
                            for r in range(R):
                                nc.tensor.matmul(
                                    acc[0:64, ts(s, 512)],
                                    wt_sb[:, r, :],
                                    xt4[:, ts(s, 512), r],
                                    start=(r == 0),
                                    stop=(r == R - 1),
                                )
                        logT = lpool.tile([64, 1024], f32, tag="logT")
                        nc.vector.tensor_copy(logT[:], acc[0:64, :])
                        for j in range(2 * SUBS):
                            n = gg * 2 * SUBS + j
                            lg_ps = lgppool.tile([128, E], f32, tag="lgps")
                            nc.tensor.transpose(
                                lg_ps[:], logT[:, ts(j, 128)], ident[:64, :64]
                            )
                            nc.vector.max(top8_all[:, n, :], lg_ps[:])
                            nc.vector.max_index(
                                idx_all[:, n, :], top8_all[:, n, :], lg_ps[:]
                            )
                        continue
                    for s in range(2):
                        lps = mmpool.tile([128, 512], f32, tag=f"lps{s}")
                        for r in range(R):
                            nc.tensor.matmul(
                                lps[0:64, :],
                                wt_sb[:, r, :],
                                xt4[:, ts(s, 512), r],
                                start=(r == 0),
                                stop=(r == R - 1),
                            )
                        logT = lpool.tile([64, 512], f32)
                        nc.vector.tensor_copy(logT[:], lps[0:64, :])
                        if v5:
                            # strided write into token-major DRAM scratch:
                            # zscratch[t, e] <- logT[e, t-slice]
                            tok0 = (2 * gg + s) * 512
                            nc.sync.dma_start(
                                zscratch[tok0 : tok0 + 512, :].rearrange(
                                    "t e -> e t"
                                ),
                                logT[:],
                            )
                            continue
                        for j in range(SUBS):
                            n = (2 * gg + s) * SUBS + j
                            lg_ps = lgppool.tile([128, E], f32, tag="lgps")
                            nc.tensor.transpose(
                                lg_ps[:], logT[:, ts(j, 128)], ident[:64, :64]
                            )
                            nc.vector.max(top8_all[:, n, :], lg_ps[:])
                            nc.vector.max_index(
                                idx_all[:, n, :], top8_all[:, n, :], lg_ps[:]
                            )
                if v5:
                    # one contiguous-last-dim read back: lg_all[p, n, e]
                    lg_all = lpool.tile([128, N_TILES, E], f32, tag="lgall")
                    nc.sync.dma_start(
                        lg_all[:],
                        zscratch.rearrange("(n p) e -> p n e", p=128),
                    )
                    for n in range(N_TILES):
                        nc.vector.max(top8_all[:, n, :], lg_all[:, n, :])
                        nc.vector.max_index(
                            idx_all[:, n, :], top8_all[:, n, :], lg_all[:, n, :]
                        )
                _epilogue(top8_all, rw_all)
                nc.sync.dma_start(
                    comb.rearrange("(a p) k -> p a k", p=128), comb_all[:]
                )
                return

            for g in range(GROUPS):
                if dload:
                    # One interleaved-transposed DMA per 512-token group:
                    # xt4[q, t, r] = x[512g + t, R*q + r].  Each matmul below
                    # contracts over the stride-R h-subset {R*q + r}.
                    xt4 = xpool.tile([128, 512, R], f32, tag="xt4")
                    nc.sync.dma_start(
                        xt4[:],
                        x[ts(g, 512), :].rearrange("t (q r) -> q t r", r=R),
                    )
                else:
                    xsub = []
                    for j in range(SUBS):
                        xs = xpool.tile([128, H], f32, tag="xs")
                        nc.sync.dma_start(xs[:], x[ts(g * SUBS + j, 128), :])
                        xsub.append(xs)

                # Even chunks accumulate into bank A partitions 0-63 (PE
                # column groups 0-1), odd chunks into bank B partitions
                # 64-127 (column groups 2-3) so the two matmul streams can
                # run concurrently on disjoint column groups of the PE array.
                lpsA = mmpool.tile([128, 512], f32, tag="lpsA")
                if pack:
                    lpsB = mmpool.tile([128, 512], f32, tag="lpsB")
                if dload:
                    for r in range(R):
                        if pack:
                            out_ps = lpsA[0:64, :] if r % 2 == 0 else lpsB[64:128, :]
                            start, stop = r < 2, r >= R - 2
                        else:
                            out_ps = lpsA[0:64, :]
                            start, stop = r == 0, r == R - 1
                        nc.tensor.matmul(
                            out_ps,
                            wt_sb[:, r, :],
                            xt4[:, :, r],
                            start=start,
                            stop=stop,
                        )
                else:
                    for c in range(CHUNKS):
                        xt_ps = ppool.tile([128, 512], f32, tag="xtps")
                        for j in range(SUBS):
                            nc.tensor.transpose(
                                xt_ps[:, ts(j, 128)], xsub[j][:, ts(c, 128)], ident[:]
                            )
                        xt_sb = xtpool.tile([128, 512], mm_dt, tag="xt")
                        nc.vector.tensor_copy(xt_sb[:], xt_ps[:])
                        if pack:
                            out_ps = lpsA[0:64, :] if c % 2 == 0 else lpsB[64:128, :]
                            nc.tensor.matmul(
                                out_ps,
                                wt_mm[:, c, :],
                                xt_sb[:],
                                start=(c < 2),
                                stop=(c >= CHUNKS - 2),
                            )
                        else:
                            nc.tensor.matmul(
                                lpsA[0:64, :],
                                wt_mm[:, c, :],
                                xt_sb[:],
                                start=(c == 0),
                                stop=(c == CHUNKS - 1),
                            )

                # only one DVE input may be PSUM: copy then add
                logT = lpool.tile([64, 512], f32)
                if pack:
                    nc.vector.tensor_copy(logT[:], lpsA[0:64, :])
                    nc.vector.tensor_add(logT[:], logT[:], lpsB[64:128, :])
                else:
                    nc.vector.tensor_copy(logT[:], lpsA[0:64, :])

                for j in range(SUBS):
                    n = g * SUBS + j
                    lg_ps = lgppool.tile([128, E], f32, tag="lgps")
                    nc.tensor.transpose(lg_ps[:], logT[:, ts(j, 128)], ident[:64, :64])
                    if dload:
                        # max8/max_index read straight from PSUM (1 PSUM input)
                        nc.vector.max(top8_all[:, n, :], lg_ps[:])
                        nc.vector.max_index(idx_all[:, n, :], top8_all[:, n, :], lg_ps[:])
                    else:
                        lg_sb = epool.tile([128, E], f32, tag="lg")
                        nc.vector.tensor_copy(lg_sb[:], lg_ps[:])
                        nc.vector.max(top8_all[:, n, :], lg_sb[:])
                        nc.vector.max_index(idx_all[:, n, :], top8_all[:, n, :], lg_sb[:])

            # Batched epilogue: routing weights from top-8 raw logits.
            # capped = 30*tanh(u/30); rw_k = exp(capped_k)/sum_j exp(capped_j)
            # (the full-softmax denominator cancels after renormalization).
            # tanh(v) = v*(1 + c3 v^2 + c5 v^4 + c7 v^6); |v| <= ~0.2 here so
            # the degree-7 truncation error is ~1e-9.
            F = N_TILES * TOPK  # 128
            u = top8_all[:].rearrange("p a b -> p (a b)")
            c3, c5, c7 = -1.0 / 3.0, 2.0 / 15.0, -17.0 / 315.0
            inv_cap = 1.0 / SOFTCAP

            v = epool.tile([128, F], f32, tag="v")
            h = epool.tile([128, F], f32, tag="h")
            p = epool.tile([128, F], f32, tag="p")
            et = epool.tile([128, N_TILES, TOPK], f32, tag="et")
            s = epool.tile([128, N_TILES, 1], f32, tag="s")
            r = epool.tile([128, N_TILES, 1], f32, tag="r")

            if lean:
                # Fold 1/30^2 into the coefficients; Horner via
                # scalar_tensor_tensor: b = (b + s) * v each step.
                sc2 = inv_cap * inv_cap
                c3s, c5s, c7s = c3 * sc2, c5 * sc2 * sc2, c7 * sc2 * sc2 * sc2
                nc.vector.tensor_mul(v[:], u, u)  # v = u^2
                nc.vector.scalar_tensor_tensor(
                    h[:], v[:], c5s / c7s, v[:],
                    op0=mybir.AluOpType.add, op1=mybir.AluOpType.mult,
                )
                nc.vector.scalar_tensor_tensor(
                    h[:], h[:], c3s / c7s, v[:],
                    op0=mybir.AluOpType.add, op1=mybir.AluOpType.mult,
                )
                # h = h*c7s + 1;  p = h*u = 30*tanh(u/30)
                nc.vector.tensor_scalar(
                    h[:], h[:], c7s, 1.0,
                    op0=mybir.AluOpType.mult, op1=mybir.AluOpType.add,
                )
                nc.vector.tensor_mul(p[:], h[:], u)
            else:
                # v = (u/30)^2
                nc.vector.tensor_mul(v[:], u, u)
                nc.vector.tensor_scalar_mul(v[:], v[:], inv_cap * inv_cap)
                # h = ((c7 v + c5) v + c3) v + 1
                nc.vector.tensor_scalar(
                    h[:], v[:], c7, c5, op0=mybir.AluOpType.mult, op1=mybir.AluOpType.add
                )
                nc.vector.tensor_mul(h[:], h[:], v[:])
                nc.vector.tensor_scalar_add(h[:], h[:], c3)
                nc.vector.tensor_mul(h[:], h[:], v[:])
                nc.vector.tensor_scalar_add(h[:], h[:], 1.0)
                # p = u * h = 30*tanh(u/30); et = exp(p)
                nc.vector.tensor_mul(p[:], h[:], u)
            et_flat = et[:].rearrange("p a b -> p (a b)")
            nc.scalar.activation(et_flat, p[:], mybir.ActivationFunctionType.Exp)
            nc.vector.reduce_sum(s[:], et[:], axis=mybir.AxisListType.X)
            nc.vector.reciprocal(r[:], s[:])
            nc.vector.tensor_mul(
                rw_all[:], et[:], r[:].to_broadcast([128, N_TILES, TOPK])
            )

            nc.sync.dma_start(rw.rearrange("(a p) k -> p a k", p=128), rw_all[:])
            nc.sync.dma_start(idx.rearrange("(a p) k -> p a k", p=128), idx_all[:])

        for _ in range(reps):
            one_pass()

    nc.compile()
    return nc


def _build7(variant, reps, nox):
    """dload7: mantissa-encoded top-k (no max_index), fused encode+drain,
    host-pretransposed contiguous input loads, weight-reuse matmul order.

    Index encoding: logits' low 6 mantissa bits are overwritten with the
    expert id BEFORE selection (q = (logit & ~63) | e on the u32 view).
    This perturbs each logit by <= 2^-17 relative -- far below the top-8
    decision margins -- and makes max8 return value+index in one op; the
    index is recovered by (q & 63).  Eliminates all 16 max_index
    instructions.

    Transpose routes: "-pe" = 16 PE transposes (default), "-v5" = DRAM
    round trip (2 DMAs).
    """
    import concourse.bass as bass
    import concourse.mybir as mybir
    import concourse.tile as tile
    from concourse import bacc
    from concourse.bass import ts
    from concourse.masks import make_identity
    from contextlib import ExitStack

    f32 = mybir.dt.float32
    u32 = mybir.dt.uint32
    route = "v5" if "-v5" in variant else "pe"
    slicewise = variant.startswith(("dload8", "dload9"))
    quarters = slicewise and ("q" in variant[6:] or variant.startswith("dload9"))
    pair = variant.startswith("dload9")
    v5route = slicewise and "v" in variant[6:]
    fine = v5route and "f" in variant[6:]      # per-slice round-trip writes
    oneop = slicewise and "s" in variant[6:]   # single-instruction encode

    nc = bacc.Bacc("TRN2", target_bir_lowering=False, debug=False)
    if nox:
        x = nc.dram_tensor("x", [128, 32, T_LOCAL], f32).ap()
    else:
        x = nc.dram_tensor("x", [128, 32, T_LOCAL], f32, kind="ExternalInput").ap()
    wt = nc.dram_tensor("wt", [H, E], f32, kind="ExternalInput").ap()
    cmask_d = nc.dram_tensor("cmask", [128, 4224], u32, kind="ExternalInput").ap()
    comb = nc.dram_tensor("out", [T_LOCAL, 2 * TOPK], u32, kind="ExternalOutput").ap()
    use_zs = route == "v5" or (slicewise and "v" in variant[6:])

    with tile.TileContext(nc) as tc, ExitStack() as ctx:
        consts = ctx.enter_context(tc.tile_pool(name="consts", bufs=1))
        xpool = ctx.enter_context(
            tc.tile_pool(name="xin", bufs=2 if (slicewise and quarters) else 1)
        )
        mmpool = ctx.enter_context(
            tc.tile_pool(name="pmm", bufs=2 if slicewise else 1, space="PSUM")
        )
        lpool = ctx.enter_context(tc.tile_pool(name="logT", bufs=2))
        epool = ctx.enter_context(tc.tile_pool(name="epi", bufs=3))
        stage = ctx.enter_context(tc.tile_pool(name="stage", bufs=2))
        if use_zs:
            zpool = ctx.enter_context(tc.tile_pool(name="zs", bufs=2, space="DRAM"))
        need_pe = (route == "pe") and not (slicewise and "v" in variant[6:])
        if need_pe:
            lgppool = ctx.enter_context(
                tc.tile_pool(name="plg", bufs=1 if slicewise else 4, space="PSUM")
            )

        wt_sb = consts.tile([128, 32, E], f32)
        nc.sync.dma_start(wt_sb[:], wt.rearrange("(q r) e -> q r e", r=32))
        cmask = consts.tile([128, 4224], u32)
        nc.sync.dma_start(cmask[:], cmask_d)
        andm = cmask[0:64, 0:2048]
        orm = cmask[0:64, 2048:4096]
        m63 = cmask[:, 4096:4224].rearrange("p (a k) -> p a k", k=TOPK)
        if need_pe:
            ident = consts.tile([128, 128], f32)
            make_identity(nc, ident[:])

        def one_pass(rep_i):
            comb_all = stage.tile([128, N_TILES, 2 * TOPK], u32, tag="comb")
            idx_all = comb_all[:, :, TOPK : 2 * TOPK]
            rw_all = comb_all[:, :, 0:TOPK].bitcast(f32)
            top8q = stage.tile([128, N_TILES, TOPK], f32, tag="top8")

            if slicewise:
                # dload8*: dload3's proven slice-by-slice structure, with the
                # fused encode-drain and no max_index.
                # q: quarter loads (double-buffered); p: pair r-outer mm;
                # v: DRAM round-trip transpose instead of PE transposes.
                logT_all = (
                    lpool.tile([64, 2048], u32, tag="logTall", name="logT_all")
                    if v5route
                    else None
                )

                zs_t = (
                    zpool.tile([T_LOCAL, E], u32, tag="zs", name="zs_t")
                    if v5route
                    else None
                )

                def encode(src_ps, g):
                    if v5route:
                        logT_u = logT_all[:, ts(g, 512)]
                    else:
                        logT_t = lpool.tile([64, 512], u32, tag="logT")
                        logT_u = logT_t[:]
                    if oneop:
                        # (acc & mask) | e in one DVE op; mask as [64,1]
                        # per-partition scalar AP, e-pattern as in1
                        nc.vector.scalar_tensor_tensor(
                            logT_u,
                            src_ps[0:64, :].bitcast(u32),
                            andm[:, 0:1],
                            orm[:, 0:512],
                            op0=mybir.AluOpType.bitwise_and,
                            op1=mybir.AluOpType.bitwise_or,
                        )
                    else:
                        tmp_u = lpool.tile([64, 512], u32, tag="tmp")
                        nc.vector.tensor_tensor(
                            tmp_u[:],
                            src_ps[0:64, :].bitcast(u32),
                            andm[:, 0:512],
                            op=mybir.AluOpType.bitwise_and,
                        )
                        nc.vector.tensor_tensor(
                            logT_u,
                            tmp_u[:],
                            orm[:, 0:512],
                            op=mybir.AluOpType.bitwise_or,
                        )
                    if fine:
                        nc.sync.dma_start(
                            zs_t[ts(g, 512), :].rearrange("t e -> e t"), logT_u
                        )
                    return logT_u

                def select(logT_u, g):
                    logT_f = logT_u.bitcast(f32)
                    for j in range(SUBS):
                        n = g * SUBS + j
                        lg_ps = lgppool.tile([128, E], f32, tag="lgps")
                        nc.tensor.transpose(
                            lg_ps[:], logT_f[:, ts(j, 128)], ident[0:64, 0:64]
                        )
                        nc.vector.max(top8q[:, n, :], lg_ps[:])

                if pair:
                    for h in range(2):
                        xts, lpss = [], []
                        for s in range(2):
                            xt = xpool.tile([128, 32, 512], f32, tag="xt")
                            nc.sync.dma_start(xt[:], x[:, :, ts(2 * h + s, 512)])
                            xts.append(xt)
                            lpss.append(
                                mmpool.tile(
                                    [128, 512], f32, tag=f"lps{s}", name=f"lps{s}"
                                )
                            )
                        for r in range(32):
                            for s in range(2):
                                nc.tensor.matmul(
                                    lpss[s][0:64, :],
                                    wt_sb[:, r, :],
                                    xts[s][:, r, :],
                                    start=(r == 0),
                                    stop=(r == 31),
                                )
                        for s in range(2):
                            g = 2 * h + s
                            lu = encode(lpss[s], g)
                            if not v5route:
                                select(lu, g)
                else:
                    for g in range(4):
                        if quarters:
                            xt = xpool.tile([128, 32, 512], f32, tag="xt")
                            nc.sync.dma_start(xt[:], x[:, :, ts(g, 512)])
                            rhs = xt
                        else:
                            if g % 2 == 0:
                                xth = xpool.tile([128, 32, 1024], f32, tag="xt")
                                nc.sync.dma_start(
                                    xth[:], x[:, :, ts(g // 2, 1024)]
                                )
                            rhs = None
                        lps = mmpool.tile([128, 512], f32, tag=f"lps{g % 2}")
                        for r in range(32):
                            nc.tensor.matmul(
                                lps[0:64, :],
                                wt_sb[:, r, :],
                                xt[:, r, :]
                                if quarters
                                else xth[:, r, ts(g % 2, 512)],
                                start=(r == 0),
                                stop=(r == 31),
                            )
                        lu = encode(lps, g)
                        if not v5route:
                            select(lu, g)

                if v5route:
                    if not fine:
                        nc.sync.dma_start(
                            zs_t.rearrange("t e -> e t"), logT_all[:]
                        )
                    lg_all = lpool.tile([128, N_TILES, E], u32, tag="lgall")
                    nc.sync.dma_start(
                        lg_all[:], zs_t.rearrange("(a p) e -> p a e", p=128)
                    )
                    for n in range(N_TILES):
                        nc.vector.max(top8q[:, n, :], lg_all.bitcast(f32)[:, n, :])
            else:
                # dload7: 2 halves, r-outer over the half's 2 token groups
                acc = mmpool.tile([128, 2048], f32, tag="acc")
                for h in range(2):
                    xt = xpool.tile([128, 32, 1024], f32, tag="xt")
                    nc.sync.dma_start(xt[:], x[:, :, ts(h, 1024)])
                    for r in range(32):
                        for s in range(2):
                            g = 2 * h + s
                            nc.tensor.matmul(
                                acc[0:64, ts(g, 512)],
                                wt_sb[:, r, :],
                                xt[:, r, ts(s, 512)],
                                start=(r == 0),
                                stop=(r == 31),
                                skip_group_check=True,
                            )

                # fused drain + index-encode: logT = (acc & ~63) | e
                tmp_u = lpool.tile([64, 2048], u32, tag="tmp")
                logT_u = lpool.tile([64, 2048], u32, tag="logT")
                nc.vector.tensor_tensor(
                    tmp_u[:],
                    acc[0:64, :].bitcast(u32),
                    andm,
                    op=mybir.AluOpType.bitwise_and,
                )
                nc.vector.tensor_tensor(
                    logT_u[:], tmp_u[:], orm, op=mybir.AluOpType.bitwise_or
                )

                if route == "v5":
                    zs_t = zpool.tile([T_LOCAL, E], u32, tag="zs", name="zs_t")
                    zs = zs_t
                    nc.sync.dma_start(zs.rearrange("t e -> e t"), logT_u[:])
                    lg_all = lpool.tile([128, N_TILES, E], u32, tag="lgall")
                    nc.sync.dma_start(
                        lg_all[:], zs.rearrange("(a p) e -> p a e", p=128)
                    )
                    for n in range(N_TILES):
                        nc.vector.max(top8q[:, n, :], lg_all.bitcast(f32)[:, n, :])
                else:
                    logT_f = logT_u[:].bitcast(f32)
                    for j4 in range(4):
                        lgps = []
                        for jj in range(4):
                            j = 4 * j4 + jj
                            lg_ps = lgppool.tile([128, E], f32, tag="lgps")
                            nc.tensor.transpose(
                                lg_ps[:], logT_f[:, ts(j, 128)], ident[0:64, 0:64]
                            )
                            lgps.append(lg_ps)
                        for jj in range(4):
                            n = 4 * j4 + jj
                            nc.vector.max(top8q[:, n, :], lgps[jj][:])

            # index decode: idx = q & 63
            nc.vector.tensor_tensor(
                idx_all,
                top8q[:].bitcast(u32),
                m63,
                op=mybir.AluOpType.bitwise_and,
            )

            # routing weights from encoded top-8 logits (see dload3 epilogue)
            F = N_TILES * TOPK
            u = top8q[:].rearrange("p a b -> p (a b)")
            c3, c5, c7 = -1.0 / 3.0, 2.0 / 15.0, -17.0 / 315.0
            sc2 = (1.0 / SOFTCAP) ** 2
            c3s, c5s, c7s = c3 * sc2, c5 * sc2 * sc2, c7 * sc2 * sc2 * sc2

            v = epool.tile([128, F], f32, tag="v")
            hh = epool.tile([128, F], f32, tag="hh")
            p = epool.tile([128, F], f32, tag="p")
            et = epool.tile([128, N_TILES, TOPK], f32, tag="et")
            s_ = epool.tile([128, N_TILES, 1], f32, tag="s")
            rcp = epool.tile([128, N_TILES, 1], f32, tag="rcp")

            nc.vector.tensor_mul(v[:], u, u)
            nc.vector.scalar_tensor_tensor(
                hh[:], v[:], c5s / c7s, v[:],
                op0=mybir.AluOpType.add, op1=mybir.AluOpType.mult,
            )
            nc.vector.scalar_tensor_tensor(
                hh[:], hh[:], c3s / c7s, v[:],
                op0=mybir.AluOpType.add, op1=mybir.AluOpType.mult,
            )
            nc.vector.tensor_scalar(
                hh[:], hh[:], c7s, 1.0,
                op0=mybir.AluOpType.mult, op1=mybir.AluOpType.add,
            )
            nc.vector.tensor_mul(p[:], hh[:], u)
            et_flat = et[:].rearrange("p a b -> p (a b)")
            nc.scalar.activation(et_flat, p[:], mybir.ActivationFunctionType.Exp)
            nc.vector.reduce_sum(s_[:], et[:], axis=mybir.AxisListType.X)
            nc.vector.reciprocal(rcp[:], s_[:])
            nc.vector.tensor_mul(
                rw_all[:], et[:], rcp[:].to_broadcast([128, N_TILES, TOPK])
            )

            nc.sync.dma_start(
                comb.rearrange("(a p) k -> p a k", p=128), comb_all[:]
            )

        for rep_i in range(reps):
            one_pass(rep_i)

    nc.compile()
    return nc


def _aux_inputs(variant):
    """Constant auxiliary inputs required by a variant (beyond x/wt)."""
    if not variant.startswith(("dload7", "dload8", "dload9")):
        return {}
    cm = np.zeros((128, 4224), np.uint32)
    cm[0:64, 0:2048] = np.uint32(0xFFFFFFC0)
    cm[0:64, 2048:4096] = np.arange(64, dtype=np.uint32)[:, None]
    cm[:, 4096:4224] = 63
    return {"cmask": cm}


def _get_nc(variant, reps=1):
    key = (variant, reps)
    if key not in _CACHE:
        _CACHE[key] = _build(variant, reps)
    return _CACHE[key]


def kernel(hidden_states, gate_weight):
    from concourse.bass_utils import run_bass_kernel_spmd

    x = np.ascontiguousarray(np.asarray(hidden_states, dtype=np.float32)).reshape(
        T_FULL, H
    )
    w = np.asarray(gate_weight, dtype=np.float32)
    wt = np.ascontiguousarray(w.T)

    variant = _variant()
    nc = _get_nc(variant, int(os.environ.get("MOE_REPS", "1")))
    aux = _aux_inputs(variant)
    if variant.startswith(("dload7", "dload8", "dload9")):
        # host pre-transpose: xd[q, r, t] = x[t, 32q + r] (contiguous loads)
        in_maps = [
            {
                "x": np.ascontiguousarray(
                    x[i * T_LOCAL : (i + 1) * T_LOCAL]
                    .reshape(T_LOCAL, 128, 32)
                    .transpose(1, 2, 0)
                ),
                "wt": wt,
                **aux,
            }
            for i in range(N_CORES)
        ]
    else:
        in_maps = [
            {"x": np.ascontiguousarray(x[i * T_LOCAL : (i + 1) * T_LOCAL]), "wt": wt}
            for i in range(N_CORES)
        ]
    kwargs = {}
    if os.environ.get("MOE_TRACE"):
        kwargs["trace"] = True
        if os.environ.get("MOE_TMPDIR"):
            kwargs["tmpdir"] = os.environ["MOE_TMPDIR"]
    res = run_bass_kernel_spmd(nc, in_maps, core_ids=list(range(N_CORES)), **kwargs)
    if os.environ.get("MOE_TRACE"):
        kernel.last_exec_time_ns = res.exec_time_ns
        kernel.last_profile = res.profile_json
    if variant.startswith(("dload3", "dload4", "dload5", "dload7", "dload8", "dload9")):
        out = np.concatenate([res.results[i]["out"] for i in range(N_CORES)], axis=0)
        rw = np.ascontiguousarray(out[:, :TOPK]).view(np.float32)
        idx = out[:, TOPK:]
    else:
        rw = np.concatenate([res.results[i]["rw"] for i in range(N_CORES)], axis=0)
        idx = np.concatenate([res.results[i]["idx"] for i in range(N_CORES)], axis=0)
    return rw.astype(np.float32), idx.astype(np.int32)

